# revision 2
# baseline (speedup 1.0000x reference)
"""Trainium2 Bass kernel v2 for windowed (inverted-window) attention.

Problem: B=2, T=2048, C=2048, H=16 heads, D=128, WINDOW=512.
  q,k,v = x@Wq, x@Wk, x@Wv  (per-head reshape), RoPE on q,k,
  scores masked so positions INSIDE the causal window are masked out
  (attend only to j>i or j<i-511), softmax, o@Wo.

Sharding: 8 cores = 2 (batch) x 4 (head groups of 4 heads).
Each core computes its batch's 4 heads end-to-end plus a partial
output projection (row-chunk of Wo); host sums the 4 partials per batch.

v2 design vs v1:
  - per-head sequential QK accumulation chains; RoPE of head h overlaps
    chain h+1 (removes the serialized RoPE tail at the A->B boundary)
  - RoPE done by DVE reading PSUM directly with cross-partition operand
    slices (no scalar copy, no DMA half-swaps)
  - one PSUM pool with [128,2,512] tags shared across both phases
    (bank-granular start=True zeroing makes half-tile sharing safe)
  - scores matmuls skip the fully-masked i-strip of diagonal j-chunks
    (exp stays dense; mask multiply zeroes the stale strip)
  - output projection of block ib-1 interleaved as PE filler during ib
  - Z: gpsimd pair-sums + vector add tree + single ones-matmul
  - bf16 output stores on the gpsimd DMA queue; host accumulates fp32
  - phase-scoped SBUF pools (phase-B et/u tiles reuse the zone of the
    phase-A weights/x/rope temporaries)
"""

import sys
import numpy as np

for _p in ("/opt/trn_rl_repo",):
    if _p not in sys.path:
        sys.path.insert(0, _p)

import ml_dtypes  # noqa: E402

try:
    import antenv.axon_hooks  # noqa: F401
except ImportError:
    import types as _types

    _hooks = _types.ModuleType("antenv.axon_hooks")
    _hooks._hook = None
    _hooks.set_axon_ntff_profile_hook = lambda h: setattr(_hooks, "_hook", h)
    _hooks.get_axon_ntff_profile_hook = lambda: _hooks._hook
    sys.modules["antenv.axon_hooks"] = _hooks
    import antenv as _antenv

    _antenv.axon_hooks = _hooks
import concourse.bass as bass  # noqa: E402
import concourse.mybir as mybir  # noqa: E402
from concourse.bacc import Bacc  # noqa: E402
from concourse.tile import TileContext  # noqa: E402
from concourse.bass import ts, ds  # noqa: E402
from concourse.bass_utils import run_bass_kernel_spmd  # noqa: E402

B, T, C, H, D = 2, 2048, 2048, 16, 128
HL = 4                # heads per core
NCORES = 8
WINDOW = 512
ROPE_BASE = 10000.0
TB = 512              # i/t block size (matmul free dim)
NTB = T // TB         # 4
CK = C // 128         # 16 contraction chunks for projections
NTC = T // 128        # 16 j-chunks / t-chunks
MASK_OFF = 511        # trimmed mask strip: col = (i0 - j0) + MASK_OFF
MASK_W = 1664
F32 = mybir.dt.float32
BF16 = mybir.dt.bfloat16
AF = mybir.ActivationFunctionType

MM_DT = BF16
NP_MM = ml_dtypes.bfloat16

SKIP_MIN = 64         # only skip fully-masked i-strips at least this wide
SIM_SAFE = False      # memset skipped strips (needed only for CoreSim)
USE_DMA_SWAP = False  # fallback if cross-partition DVE operands are rejected

_NC = None
TRACE = False
LAST_RESULT = None


def _skip_strip(ib, c):
    """Fully-masked i-range (relative to block start) for j-chunk c in
    i-block ib: absolute i in [128c+127, 128c+511]."""
    lo = 128 * c + 127 - TB * ib
    hi = 128 * c + 512 - TB * ib
    lo, hi = max(lo, 0), min(hi, TB)
    if hi - lo >= SKIP_MIN:
        return lo, hi
    return None


def _mask_dd(ib, c):
    dd = ib * TB - c * 128
    if -(WINDOW - 1) <= dd <= (WINDOW - 1) + 127:
        return dd + MASK_OFF
    return None


def build_nc():
    nc = Bacc()
    xT = nc.declare_dram_parameter("xT", [C, T], MM_DT, isOutput=False)
    wq = nc.declare_dram_parameter("wq", [C, HL * D], MM_DT, isOutput=False)
    wk = nc.declare_dram_parameter("wk", [C, HL * D], MM_DT, isOutput=False)
    wv = nc.declare_dram_parameter("wv", [C, HL * D], MM_DT, isOutput=False)
    wo = nc.declare_dram_parameter("wo", [HL * D, C], MM_DT, isOutput=False)
    cosx = nc.declare_dram_parameter("cosx", [128, T], MM_DT, isOutput=False)
    sinx = nc.declare_dram_parameter("sinx", [128, T], MM_DT, isOutput=False)
    maskm = nc.declare_dram_parameter("maskm", [128, MASK_W], MM_DT, isOutput=False)
    out = nc.declare_dram_parameter("out", [T, C], MM_DT, isOutput=True)

    xT_v = xT[:].rearrange("(co p) t -> p co t", p=128)   # [128, 16, T]
    wq_v = wq[:].rearrange("(co p) d -> p co d", p=128)   # [128, 16, 512]
    wk_v = wk[:].rearrange("(co p) d -> p co d", p=128)
    wv_v = wv[:].rearrange("(co p) d -> p co d", p=128)
    wo_v = wo[:].rearrange("(h p) c -> p h c", p=128)     # [128, 4, C]

    scale = float(1.0 / np.sqrt(D))

    with TileContext(nc) as tc:
        with (
            tc.tile_pool(name="res", bufs=1) as res,
            tc.tile_pool(name="smp", bufs=2) as smp,
            tc.tile_pool(name="ocb", bufs=2) as ocb,
            tc.tile_pool(name="ps", bufs=1, space="PSUM") as psum,
        ):
            # ---- long-lived residents needed through phase B ----
            maskb = res.tile([128, MASK_W], MM_DT)
            wob = res.tile([128, HL, C], MM_DT)
            ones = res.tile([128, 128], MM_DT)
            qts = [res.tile([128, HL, TB], MM_DT, name=f"QT{t}") for t in range(NTB)]
            kts = [res.tile([128, HL, TB], MM_DT, name=f"KT{t}") for t in range(NTB)]
            V = res.tile([128, NTC, HL * D], MM_DT)
            oT = res.tile([128, HL, T], MM_DT)

            # 4 PSUM tile tags of [128, 2, TB] f32 (2 banks each), shared by
            # both phases.
            def ptile(i, name):
                return psum.tile([128, 2, TB], F32, tag=f"p{i}", name=name)

            # ---- Phase A: projections + RoPE (per-head chains), V ----
            with (
                tc.tile_pool(name="wp", bufs=1) as wp,
                tc.tile_pool(name="xbp", bufs=2) as xbp,
                tc.tile_pool(name="ropep", bufs=3) as ropep,
            ):
                # wk first, in ck-group chunks: the first QK chain can
                # start after the first chunk lands
                wkb = wp.tile([128, CK, HL * D], MM_DT)
                wqb = wp.tile([128, CK, HL * D], MM_DT)
                wvb = wp.tile([128, CK, HL * D], MM_DT)
                for g in range(4):
                    nc.sync.dma_start(wkb[:, ts(g, 4), :], wk_v[:, ts(g, 4), :])
                cosb = wp.tile([128, T], MM_DT)
                sinb = wp.tile([128, T], MM_DT)
                nc.sync.dma_start(cosb[:], cosx[:])
                nc.sync.dma_start(sinb[:], sinx[:])
                for g in range(4):
                    nc.sync.dma_start(wqb[:, ts(g, 4), :], wq_v[:, ts(g, 4), :])
                for g in range(2):
                    nc.sync.dma_start(wvb[:, ts(g, 8), :], wv_v[:, ts(g, 8), :])
                nc.sync.dma_start(maskb[:], maskm[:])
                nc.sync.dma_start(wob[:], wo_v[:])
                nc.gpsimd.memset(ones[:], 1.0)

                for tb in range(NTB):
                    xb = xbp.tile([128, CK, TB], MM_DT, tag="xb", name=f"xb{tb}")
                    for g in range(4):
                        nc.gpsimd.dma_start(
                            xb[:, ts(g, 4), :], xT_v[:, ts(g, 4), ts(tb, TB)]
                        )
                    for h in range(HL):
                        ph = ptile(h, f"pqk{tb}_{h}")
                        # K chain then Q chain (phase B scores of head h need
                        # K of the last tb first)
                        for half, wb in ((1, wkb), (0, wqb)):
                            for ck in range(CK):
                                nc.tensor.matmul(
                                    ph[:, half, :], wb[:, ck, ts(h, D)],
                                    xb[:, ck, :],
                                    start=(ck == 0), stop=(ck == CK - 1),
                                )
                        for half, OUTT in ((1, kts[tb]), (0, qts[tb])):
                            ps = ph[:, half, :]
                            if not USE_DMA_SWAP:
                                # RoPE on DVE reading PSUM directly:
                                #   out[0:64]   = ps[0:64]*cos - ps[64:128]*sin
                                #   out[64:128] = ps[64:128]*cos + ps[0:64]*sin
                                # sinx rows hold [-sin; sin]: both halves add.
                                tsw = ropep.tile([128, TB], F32, tag="tsw")
                                nc.vector.tensor_mul(
                                    tsw[0:64, :], ps[64:128, :],
                                    sinb[0:64, ts(tb, TB)],
                                )
                                nc.vector.tensor_mul(
                                    tsw[64:128, :], ps[0:64, :],
                                    sinb[64:128, ts(tb, TB)],
                                )
                                tcs = ropep.tile([128, TB], F32, tag="tcs")
                                nc.vector.tensor_mul(
                                    tcs[:], ps[:], cosb[:, ts(tb, TB)]
                                )
                                nc.vector.tensor_add(
                                    OUTT[:, h, :], tsw[:], tcs[:]
                                )
                            else:
                                sw = ropep.tile([128, TB], F32, tag="tsw")
                                nc.scalar.dma_start(sw[0:64, :], ps[64:128, :])
                                nc.scalar.dma_start(sw[64:128, :], ps[0:64, :])
                                nc.vector.tensor_mul(
                                    sw[:], sw[:], sinb[:, ts(tb, TB)]
                                )
                                tcs = ropep.tile([128, TB], F32, tag="tcs")
                                nc.vector.tensor_mul(
                                    tcs[:], ps[:], cosb[:, ts(tb, TB)]
                                )
                                nc.vector.tensor_add(OUTT[:, h, :], sw[:], tcs[:])
                    # V for the 4 t-chunks of this t-block
                    for tco in range(NTB):
                        tch = tb * NTB + tco
                        pv = ptile(tco // 2, f"pv{tb}_{tco}")[:, tco % 2, :]
                        for ck in range(CK):
                            nc.tensor.matmul(
                                pv[:], xb[:, ck, ts(tco, 128)], wvb[:, ck, :],
                                start=(ck == 0), stop=(ck == CK - 1),
                            )
                        nc.scalar.copy(V[:, tch, :], pv[:])

            # ---- Phase B: attention, with O-proj of ib-1 interleaved ----
            with (
                tc.tile_pool(name="etp", bufs=10) as etp,
                tc.tile_pool(name="up", bufs=2) as up,
            ):
                def oproj_group(ibo, cb, tags=(3,)):
                    for tto in range(NTB):
                        tt = ibo * NTB + tto
                        tg = tags[tto % len(tags)]
                        pc = ptile(tg, f"po{ibo}_{cb}_{tto}")[:, (tto // len(tags)) % 2, :]
                        for hh in range(HL):
                            nc.tensor.matmul(
                                pc[:], oT[:, hh, ts(tt, 128)],
                                wob[:, hh, ds(cb * TB, TB)],
                                start=(hh == 0), stop=(hh == HL - 1),
                            )
                        ob = ocb.tile([128, TB], MM_DT, tag=f"ob{tto % 2}")
                        if tto % 2 == 0:
                            nc.scalar.copy(ob[:], pc[:])
                        else:
                            nc.vector.tensor_copy(ob[:], pc[:])
                        nc.sync.dma_start(
                            out[ts(tt, 128), ds(cb * TB, TB)], ob[:]
                        )

                for ib in range(NTB):
                    for h in range(HL):
                        ets = []
                        u = up.tile([128, NTC // 2, TB], MM_DT, tag="u",
                                    name=f"u{ib}_{h}")
                        for cp in range(NTC // 2):
                            sp = ptile(cp % 2, f"ps{ib}_{h}_{cp}")
                            for k in range(2):
                                c = 2 * cp + k
                                kt = kts[c // 4][:, h, ts(c % 4, 128)]
                                strip = _skip_strip(ib, c)
                                if strip is None:
                                    nc.tensor.matmul(
                                        sp[:, k, :], kt, qts[ib][:, h, :],
                                        start=True, stop=True,
                                    )
                                else:
                                    lo, hi = strip
                                    if SIM_SAFE:
                                        # defined data for CoreSim's race check;
                                        # on HW the stale strip is bounded and
                                        # the mask multiply zeroes it
                                        nc.vector.memset(sp[:, k, lo:hi], 0.0)
                                    first = True
                                    if lo > 0:
                                        nc.tensor.matmul(
                                            sp[:, k, 0:lo], kt,
                                            qts[ib][:, h, 0:lo],
                                            start=first, stop=True,
                                        )
                                        first = False
                                    if hi < TB:
                                        nc.tensor.matmul(
                                            sp[:, k, hi:TB], kt,
                                            qts[ib][:, h, hi:TB],
                                            start=first, stop=True,
                                        )
                            et = etp.tile([128, 2, TB], MM_DT, tag="et")
                            nc.scalar.activation(et[:], sp[:], AF.Exp, scale=scale)
                            for k in range(2):
                                c = 2 * cp + k
                                off = _mask_dd(ib, c)
                                if off is not None:
                                    nc.vector.tensor_mul(
                                        et[:, k, :], et[:, k, :],
                                        maskb[:, ds(off, TB)],
                                    )
                            eng = nc.gpsimd if cp < 4 else nc.vector
                            eng.tensor_add(
                                u[:, cp, :], et[:, 0, :], et[:, 1, :]
                            )
                            ets.append(et)
                        # AV accumulation (dense; masked strips are 0 in et)
                        pso = ptile(2, f"pso{ib}_{h}")
                        for c in range(NTC):
                            nc.tensor.matmul(
                                pso[:, 0, :], V[:, c, ts(h, D)],
                                ets[c // 2][:, c % 2, :],
                                start=(c == 0), stop=(c == NTC - 1),
                            )
                        # O-proj filler first: keeps the PE busy while the
                        # Z add tree drains (in-order engine queue)
                        if ib > 0:
                            oproj_group(ib - 1, h)
                        # Z: add tree 8 -> 1 on vector, then ones-matmul
                        for k in range(4):
                            nc.vector.tensor_add(
                                u[:, k, :], u[:, k, :], u[:, k + 4, :]
                            )
                        nc.vector.tensor_add(u[:, 0, :], u[:, 0, :], u[:, 1, :])
                        nc.vector.tensor_add(u[:, 2, :], u[:, 2, :], u[:, 3, :])
                        nc.vector.tensor_add(u[:, 0, :], u[:, 0, :], u[:, 2, :])
                        nc.tensor.matmul(
                            pso[:, 1, :], ones[:], u[:, 0, :],
                            start=True, stop=True,
                        )
                        rz = smp.tile([128, TB], F32, tag="rz")
                        nc.vector.reciprocal_approx_fast(rz[:], pso[:, 1, :])
                        nc.vector.tensor_mul(
                            oT[:, h, ts(ib, TB)], pso[:, 0, :], rz[:]
                        )
                for cb in range(NTB):
                    oproj_group(NTB - 1, cb, tags=(0, 1, 2, 3))

    nc.finalize()
    return nc


def _host_tables():
    inv_freq = (
        1.0 / (np.float32(ROPE_BASE) ** (np.arange(0, D, 2, dtype=np.float32) / np.float32(D)))
    ).astype(np.float32)
    t = np.arange(T, dtype=np.float32)
    freqs = (t[:, None] * inv_freq[None, :]).astype(np.float32)  # [T, 64]
    cos = np.cos(freqs).T.astype(np.float32)                     # [64, T]
    sin = np.sin(freqs).T.astype(np.float32)
    cosx = np.ascontiguousarray(np.concatenate([cos, cos], axis=0)).astype(NP_MM)
    sinx = np.ascontiguousarray(np.concatenate([-sin, sin], axis=0)).astype(NP_MM)
    p = np.arange(128, dtype=np.int64)[:, None]
    u = np.arange(MASK_W, dtype=np.int64)[None, :]
    delta = u - MASK_OFF - p          # = i - j for tile offset
    allow = ~((delta >= 0) & (delta <= WINDOW - 1))
    maskm = np.ascontiguousarray(allow.astype(NP_MM))
    return cosx, sinx, maskm


def make_in_maps(x, Wq, Wk, Wv, Wo):
    cosx, sinx, maskm = _host_tables()
    in_maps = []
    for core in range(NCORES):
        b, hg = divmod(core, NCORES // B)
        sl = slice(hg * HL * D, (hg + 1) * HL * D)
        in_maps.append(
            {
                "xT": np.ascontiguousarray(x[b].T.astype(NP_MM)),
                "wq": np.ascontiguousarray(Wq[:, sl].astype(NP_MM)),
                "wk": np.ascontiguousarray(Wk[:, sl].astype(NP_MM)),
                "wv": np.ascontiguousarray(Wv[:, sl].astype(NP_MM)),
                "wo": np.ascontiguousarray(Wo[sl, :].astype(NP_MM)),
                "cosx": cosx,
                "sinx": sinx,
                "maskm": maskm,
            }
        )
    return in_maps


def kernel(x, Wq, Wk, Wv, Wo):
    global _NC, LAST_RESULT
    if _NC is None:
        _NC = build_nc()
    x = np.asarray(x, dtype=np.float32)
    Wq = np.asarray(Wq, dtype=np.float32)
    Wk = np.asarray(Wk, dtype=np.float32)
    Wv = np.asarray(Wv, dtype=np.float32)
    Wo = np.asarray(Wo, dtype=np.float32)
    in_maps = make_in_maps(x, Wq, Wk, Wv, Wo)
    res = run_bass_kernel_spmd(_NC, in_maps, list(range(NCORES)), trace=TRACE)
    LAST_RESULT = res
    out = np.zeros((B, T, C), dtype=np.float32)
    for core in range(NCORES):
        b = core // (NCORES // B)
        out[b] += res.results[core]["out"].astype(np.float32)
    return out


# revision 3
# speedup vs baseline: 1.1512x; 1.1512x over previous
"""Trainium2 Bass kernel v2 for windowed (inverted-window) attention.

Problem: B=2, T=2048, C=2048, H=16 heads, D=128, WINDOW=512.
  q,k,v = x@Wq, x@Wk, x@Wv  (per-head reshape), RoPE on q,k,
  scores masked so positions INSIDE the causal window are masked out
  (attend only to j>i or j<i-511), softmax, o@Wo.

Sharding: 8 cores = 2 (batch) x 4 (head groups of 4 heads).
Each core computes its batch's 4 heads end-to-end plus a partial
output projection (row-chunk of Wo); host sums the 4 partials per batch.

v2 design vs v1:
  - per-head sequential QK accumulation chains; RoPE of head h overlaps
    chain h+1 (removes the serialized RoPE tail at the A->B boundary)
  - RoPE done by DVE reading PSUM directly with cross-partition operand
    slices (no scalar copy, no DMA half-swaps)
  - one PSUM pool with [128,2,512] tags shared across both phases
    (bank-granular start=True zeroing makes half-tile sharing safe)
  - scores matmuls skip the fully-masked i-strip of diagonal j-chunks
    (exp stays dense; mask multiply zeroes the stale strip)
  - output projection of block ib-1 interleaved as PE filler during ib
  - Z: gpsimd pair-sums + vector add tree + single ones-matmul
  - bf16 output stores on the gpsimd DMA queue; host accumulates fp32
  - phase-scoped SBUF pools (phase-B et/u tiles reuse the zone of the
    phase-A weights/x/rope temporaries)
"""

import sys
import numpy as np

for _p in ("/opt/trn_rl_repo",):
    if _p not in sys.path:
        sys.path.insert(0, _p)

import ml_dtypes  # noqa: E402

try:
    import antenv.axon_hooks  # noqa: F401
except ImportError:
    import types as _types

    _hooks = _types.ModuleType("antenv.axon_hooks")
    _hooks._hook = None
    _hooks.set_axon_ntff_profile_hook = lambda h: setattr(_hooks, "_hook", h)
    _hooks.get_axon_ntff_profile_hook = lambda: _hooks._hook
    sys.modules["antenv.axon_hooks"] = _hooks
    import antenv as _antenv

    _antenv.axon_hooks = _hooks
import concourse.bass as bass  # noqa: E402
import concourse.mybir as mybir  # noqa: E402
from concourse.bacc import Bacc  # noqa: E402
from concourse.tile import TileContext  # noqa: E402
from concourse.bass import ts, ds  # noqa: E402
from concourse.bass_utils import run_bass_kernel_spmd  # noqa: E402

B, T, C, H, D = 2, 2048, 2048, 16, 128
HL = 4                # heads per core
NCORES = 8
WINDOW = 512
ROPE_BASE = 10000.0
TB = 512              # i/t block size (matmul free dim)
NTB = T // TB         # 4
CK = C // 128         # 16 contraction chunks for projections
NTC = T // 128        # 16 j-chunks / t-chunks
MASK_OFF = 511        # trimmed mask strip: col = (i0 - j0) + MASK_OFF
MASK_W = 1664
F32 = mybir.dt.float32
BF16 = mybir.dt.bfloat16
AF = mybir.ActivationFunctionType

MM_DT = BF16
NP_MM = ml_dtypes.bfloat16

SKIP_MIN = 64         # only skip fully-masked i-strips at least this wide
SIM_SAFE = False      # memset skipped strips (needed only for CoreSim)
USE_DMA_SWAP = False  # fallback if cross-partition DVE operands are rejected

_NC = None
TRACE = False
LAST_RESULT = None


def _skip_strip(ib, c):
    """Fully-masked i-range (relative to block start) for j-chunk c in
    i-block ib: absolute i in [128c+127, 128c+511]."""
    lo = 128 * c + 127 - TB * ib
    hi = 128 * c + 512 - TB * ib
    lo, hi = max(lo, 0), min(hi, TB)
    if hi - lo >= SKIP_MIN:
        return lo, hi
    return None


def _mask_dd(ib, c):
    dd = ib * TB - c * 128
    if -(WINDOW - 1) <= dd <= (WINDOW - 1) + 127:
        return dd + MASK_OFF
    return None


def build_nc():
    nc = Bacc()
    xT = nc.declare_dram_parameter("xT", [C, T], MM_DT, isOutput=False)
    wq = nc.declare_dram_parameter("wq", [C, HL * D], MM_DT, isOutput=False)
    wk = nc.declare_dram_parameter("wk", [C, HL * D], MM_DT, isOutput=False)
    wv = nc.declare_dram_parameter("wv", [C, HL * D], MM_DT, isOutput=False)
    wo = nc.declare_dram_parameter("wo", [HL * D, C], MM_DT, isOutput=False)
    cosx = nc.declare_dram_parameter("cosx", [128, T], MM_DT, isOutput=False)
    sinx = nc.declare_dram_parameter("sinx", [128, T], MM_DT, isOutput=False)
    maskm = nc.declare_dram_parameter("maskm", [128, MASK_W], MM_DT, isOutput=False)
    out = nc.declare_dram_parameter("out", [T, C], MM_DT, isOutput=True)

    xT_v = xT[:].rearrange("(co p) t -> p co t", p=128)   # [128, 16, T]
    wq_v = wq[:].rearrange("(co p) d -> p co d", p=128)   # [128, 16, 512]
    wk_v = wk[:].rearrange("(co p) d -> p co d", p=128)
    wv_v = wv[:].rearrange("(co p) d -> p co d", p=128)
    wo_v = wo[:].rearrange("(h p) c -> p h c", p=128)     # [128, 4, C]

    scale = float(1.0 / np.sqrt(D))

    with TileContext(nc) as tc:
        with (
            tc.tile_pool(name="res", bufs=1) as res,
            tc.tile_pool(name="smp", bufs=2) as smp,
            tc.tile_pool(name="ocb", bufs=2) as ocb,
            tc.tile_pool(name="ps", bufs=1, space="PSUM") as psum,
        ):
            # ---- long-lived residents needed through phase B ----
            maskb = res.tile([128, MASK_W], MM_DT)
            wob = res.tile([128, HL, C], MM_DT)
            ones = res.tile([128, 128], MM_DT)
            qts = [res.tile([128, HL, TB], MM_DT, name=f"QT{t}") for t in range(NTB)]
            kts = [res.tile([128, HL, TB], MM_DT, name=f"KT{t}") for t in range(NTB)]
            V = res.tile([128, NTC, HL * D], MM_DT)
            oT = res.tile([128, HL, T], MM_DT)

            # 4 PSUM tile tags of [128, 2, TB] f32 (2 banks each), shared by
            # both phases.
            def ptile(i, name):
                return psum.tile([128, 2, TB], F32, tag=f"p{i}", name=name)

            # ---- Phase A: projections + RoPE (per-head chains), V ----
            with (
                tc.tile_pool(name="wp", bufs=1) as wp,
                tc.tile_pool(name="xbp", bufs=2) as xbp,
                tc.tile_pool(name="ropep", bufs=3) as ropep,
            ):
                # wk first, in ck-group chunks: the first QK chain can
                # start after the first chunk lands
                wkb = wp.tile([128, CK, HL * D], MM_DT)
                wqb = wp.tile([128, CK, HL * D], MM_DT)
                wvb = wp.tile([128, CK, HL * D], MM_DT)
                for a, b in ((0, 1), (1, 2), (2, 4), (4, 8), (8, 16)):
                    nc.sync.dma_start(wkb[:, a:b, :], wk_v[:, a:b, :])
                cosb = wp.tile([128, T], MM_DT)
                sinb = wp.tile([128, T], MM_DT)
                nc.sync.dma_start(cosb[:], cosx[:])
                nc.sync.dma_start(sinb[:], sinx[:])
                for g in range(4):
                    nc.sync.dma_start(wqb[:, ts(g, 4), :], wq_v[:, ts(g, 4), :])
                for g in range(2):
                    nc.sync.dma_start(wvb[:, ts(g, 8), :], wv_v[:, ts(g, 8), :])
                nc.sync.dma_start(maskb[:], maskm[:])
                nc.sync.dma_start(wob[:], wo_v[:])
                nc.gpsimd.memset(ones[:], 1.0)

                for tb in range(NTB):
                    xb = xbp.tile([128, CK, TB], MM_DT, tag="xb", name=f"xb{tb}")
                    groups = ((0, 1), (1, 2), (2, 4), (4, 8), (8, 16)) if tb == 0 \
                        else ((0, 4), (4, 8), (8, 12), (12, 16))
                    for a, b in groups:
                        nc.gpsimd.dma_start(
                            xb[:, a:b, :], xT_v[:, a:b, ts(tb, TB)]
                        )
                    for h in range(HL):
                        ph = ptile(h, f"pqk{tb}_{h}")
                        # K chain then Q chain (phase B scores of head h need
                        # K of the last tb first)
                        for half, wb in ((1, wkb), (0, wqb)):
                            for ck in range(CK):
                                nc.tensor.matmul(
                                    ph[:, half, :], wb[:, ck, ts(h, D)],
                                    xb[:, ck, :],
                                    start=(ck == 0), stop=(ck == CK - 1),
                                )
                        for half, OUTT in ((1, kts[tb]), (0, qts[tb])):
                            ps = ph[:, half, :]
                            if not USE_DMA_SWAP:
                                # RoPE on DVE reading PSUM directly:
                                #   out[0:64]   = ps[0:64]*cos - ps[64:128]*sin
                                #   out[64:128] = ps[64:128]*cos + ps[0:64]*sin
                                # sinx rows hold [-sin; sin]: both halves add.
                                tsw = ropep.tile([128, TB], F32, tag="tsw")
                                nc.vector.tensor_mul(
                                    tsw[0:64, :], ps[64:128, :],
                                    sinb[0:64, ts(tb, TB)],
                                )
                                nc.vector.tensor_mul(
                                    tsw[64:128, :], ps[0:64, :],
                                    sinb[64:128, ts(tb, TB)],
                                )
                                tcs = ropep.tile([128, TB], F32, tag="tcs")
                                nc.vector.tensor_mul(
                                    tcs[:], ps[:], cosb[:, ts(tb, TB)]
                                )
                                nc.vector.tensor_add(
                                    OUTT[:, h, :], tsw[:], tcs[:]
                                )
                            else:
                                sw = ropep.tile([128, TB], F32, tag="tsw")
                                nc.scalar.dma_start(sw[0:64, :], ps[64:128, :])
                                nc.scalar.dma_start(sw[64:128, :], ps[0:64, :])
                                nc.vector.tensor_mul(
                                    sw[:], sw[:], sinb[:, ts(tb, TB)]
                                )
                                tcs = ropep.tile([128, TB], F32, tag="tcs")
                                nc.vector.tensor_mul(
                                    tcs[:], ps[:], cosb[:, ts(tb, TB)]
                                )
                                nc.vector.tensor_add(OUTT[:, h, :], sw[:], tcs[:])
                    # V for the 4 t-chunks of this t-block
                    for tco in range(NTB):
                        tch = tb * NTB + tco
                        pv = ptile(tco // 2, f"pv{tb}_{tco}")[:, tco % 2, :]
                        for ck in range(CK):
                            nc.tensor.matmul(
                                pv[:], xb[:, ck, ts(tco, 128)], wvb[:, ck, :],
                                start=(ck == 0), stop=(ck == CK - 1),
                            )
                        nc.scalar.copy(V[:, tch, :], pv[:])

            # ---- Phase B: attention, with O-proj of ib-1 interleaved ----
            with (
                tc.tile_pool(name="etp", bufs=10) as etp,
                tc.tile_pool(name="up", bufs=2) as up,
            ):
                def oproj_group(ibo, cb, tags=(3,)):
                    for tto in range(NTB):
                        tt = ibo * NTB + tto
                        tg = tags[tto % len(tags)]
                        pc = ptile(tg, f"po{ibo}_{cb}_{tto}")[:, (tto // len(tags)) % 2, :]
                        for hh in range(HL):
                            nc.tensor.matmul(
                                pc[:], oT[:, hh, ts(tt, 128)],
                                wob[:, hh, ds(cb * TB, TB)],
                                start=(hh == 0), stop=(hh == HL - 1),
                            )
                        ob = ocb.tile([128, TB], MM_DT, tag=f"ob{tto % 2}")
                        if tto % 2 == 0:
                            nc.scalar.copy(ob[:], pc[:])
                        else:
                            nc.vector.tensor_copy(ob[:], pc[:])
                        nc.sync.dma_start(
                            out[ts(tt, 128), ds(cb * TB, TB)], ob[:]
                        )

                for ib in range(NTB):
                    for h in range(HL):
                        ets = []
                        u = up.tile([128, NTC // 2, TB], MM_DT, tag="u",
                                    name=f"u{ib}_{h}")
                        pso = ptile(2, f"pso{ib}_{h}")
                        for cp in range(NTC // 2):
                            sp = ptile(cp % 2, f"ps{ib}_{h}_{cp}")
                            for k in range(2):
                                c = 2 * cp + k
                                kt = kts[c // 4][:, h, ts(c % 4, 128)]
                                strip = _skip_strip(ib, c)
                                if strip is None:
                                    nc.tensor.matmul(
                                        sp[:, k, :], kt, qts[ib][:, h, :],
                                        start=True, stop=True,
                                    )
                                else:
                                    lo, hi = strip
                                    if SIM_SAFE:
                                        # defined data for CoreSim's race check;
                                        # on HW the stale strip is bounded and
                                        # the mask multiply zeroes it
                                        nc.vector.memset(sp[:, k, lo:hi], 0.0)
                                    first = True
                                    if lo > 0:
                                        nc.tensor.matmul(
                                            sp[:, k, 0:lo], kt,
                                            qts[ib][:, h, 0:lo],
                                            start=first, stop=True,
                                        )
                                        first = False
                                    if hi < TB:
                                        nc.tensor.matmul(
                                            sp[:, k, hi:TB], kt,
                                            qts[ib][:, h, hi:TB],
                                            start=first, stop=True,
                                        )
                            et = etp.tile([128, 2, TB], MM_DT, tag="et")
                            nc.scalar.activation(et[:], sp[:], AF.Exp, scale=scale)
                            for k in range(2):
                                c = 2 * cp + k
                                off = _mask_dd(ib, c)
                                if off is not None:
                                    nc.vector.tensor_mul(
                                        et[:, k, :], et[:, k, :],
                                        maskb[:, ds(off, TB)],
                                    )
                            eng = nc.gpsimd if cp < 4 else nc.vector
                            eng.tensor_add(
                                u[:, cp, :], et[:, 0, :], et[:, 1, :]
                            )
                            ets.append(et)
                            # interleave AV of pair cp-2 behind the scores:
                            # gives the PE work while exp catches up
                            if cp >= 2:
                                for c in (2 * cp - 4, 2 * cp - 3):
                                    nc.tensor.matmul(
                                        pso[:, 0, :], V[:, c, ts(h, D)],
                                        ets[c // 2][:, c % 2, :],
                                        start=(c == 0), stop=False,
                                    )
                        for c in range(2 * NTC // 2 - 4, NTC):
                            nc.tensor.matmul(
                                pso[:, 0, :], V[:, c, ts(h, D)],
                                ets[c // 2][:, c % 2, :],
                                start=False, stop=(c == NTC - 1),
                            )
                        # O-proj filler next: keeps the PE busy while the
                        # Z add tree drains (in-order engine queue)
                        if ib > 0:
                            oproj_group(ib - 1, h)
                        # Z: add tree 8 -> 1 on vector, then ones-matmul
                        for k in range(4):
                            nc.vector.tensor_add(
                                u[:, k, :], u[:, k, :], u[:, k + 4, :]
                            )
                        nc.vector.tensor_add(u[:, 0, :], u[:, 0, :], u[:, 1, :])
                        nc.vector.tensor_add(u[:, 2, :], u[:, 2, :], u[:, 3, :])
                        nc.vector.tensor_add(u[:, 0, :], u[:, 0, :], u[:, 2, :])
                        nc.tensor.matmul(
                            pso[:, 1, :], ones[:], u[:, 0, :],
                            start=True, stop=True,
                        )
                        rz = smp.tile([128, TB], F32, tag="rz")
                        nc.vector.reciprocal_approx_fast(rz[:], pso[:, 1, :])
                        nc.vector.tensor_mul(
                            oT[:, h, ts(ib, TB)], pso[:, 0, :], rz[:]
                        )
                for cb in range(NTB):
                    oproj_group(NTB - 1, cb, tags=(0, 1, 2, 3))

    nc.finalize()
    return nc


def _host_tables():
    inv_freq = (
        1.0 / (np.float32(ROPE_BASE) ** (np.arange(0, D, 2, dtype=np.float32) / np.float32(D)))
    ).astype(np.float32)
    t = np.arange(T, dtype=np.float32)
    freqs = (t[:, None] * inv_freq[None, :]).astype(np.float32)  # [T, 64]
    cos = np.cos(freqs).T.astype(np.float32)                     # [64, T]
    sin = np.sin(freqs).T.astype(np.float32)
    cosx = np.ascontiguousarray(np.concatenate([cos, cos], axis=0)).astype(NP_MM)
    sinx = np.ascontiguousarray(np.concatenate([-sin, sin], axis=0)).astype(NP_MM)
    p = np.arange(128, dtype=np.int64)[:, None]
    u = np.arange(MASK_W, dtype=np.int64)[None, :]
    delta = u - MASK_OFF - p          # = i - j for tile offset
    allow = ~((delta >= 0) & (delta <= WINDOW - 1))
    maskm = np.ascontiguousarray(allow.astype(NP_MM))
    return cosx, sinx, maskm


def make_in_maps(x, Wq, Wk, Wv, Wo):
    cosx, sinx, maskm = _host_tables()
    in_maps = []
    for core in range(NCORES):
        b, hg = divmod(core, NCORES // B)
        sl = slice(hg * HL * D, (hg + 1) * HL * D)
        in_maps.append(
            {
                "xT": np.ascontiguousarray(x[b].T.astype(NP_MM)),
                "wq": np.ascontiguousarray(Wq[:, sl].astype(NP_MM)),
                "wk": np.ascontiguousarray(Wk[:, sl].astype(NP_MM)),
                "wv": np.ascontiguousarray(Wv[:, sl].astype(NP_MM)),
                "wo": np.ascontiguousarray(Wo[sl, :].astype(NP_MM)),
                "cosx": cosx,
                "sinx": sinx,
                "maskm": maskm,
            }
        )
    return in_maps


def kernel(x, Wq, Wk, Wv, Wo):
    global _NC, LAST_RESULT
    if _NC is None:
        _NC = build_nc()
    x = np.asarray(x, dtype=np.float32)
    Wq = np.asarray(Wq, dtype=np.float32)
    Wk = np.asarray(Wk, dtype=np.float32)
    Wv = np.asarray(Wv, dtype=np.float32)
    Wo = np.asarray(Wo, dtype=np.float32)
    in_maps = make_in_maps(x, Wq, Wk, Wv, Wo)
    res = run_bass_kernel_spmd(_NC, in_maps, list(range(NCORES)), trace=TRACE)
    LAST_RESULT = res
    out = np.zeros((B, T, C), dtype=np.float32)
    for core in range(NCORES):
        b = core // (NCORES // B)
        out[b] += res.results[core]["out"].astype(np.float32)
    return out


# revision 4
# speedup vs baseline: 1.1591x; 1.0069x over previous
"""Trainium2 Bass kernel v2 for windowed (inverted-window) attention.

Problem: B=2, T=2048, C=2048, H=16 heads, D=128, WINDOW=512.
  q,k,v = x@Wq, x@Wk, x@Wv  (per-head reshape), RoPE on q,k,
  scores masked so positions INSIDE the causal window are masked out
  (attend only to j>i or j<i-511), softmax, o@Wo.

Sharding: 8 cores = 2 (batch) x 4 (head groups of 4 heads).
Each core computes its batch's 4 heads end-to-end plus a partial
output projection (row-chunk of Wo); host sums the 4 partials per batch.

v2 design vs v1:
  - per-head sequential QK accumulation chains; RoPE of head h overlaps
    chain h+1 (removes the serialized RoPE tail at the A->B boundary)
  - RoPE done by DVE reading PSUM directly with cross-partition operand
    slices (no scalar copy, no DMA half-swaps)
  - one PSUM pool with [128,2,512] tags shared across both phases
    (bank-granular start=True zeroing makes half-tile sharing safe)
  - scores matmuls skip the fully-masked i-strip of diagonal j-chunks
    (exp stays dense; mask multiply zeroes the stale strip)
  - output projection of block ib-1 interleaved as PE filler during ib
  - Z: gpsimd pair-sums + vector add tree + single ones-matmul
  - bf16 output stores on the gpsimd DMA queue; host accumulates fp32
  - phase-scoped SBUF pools (phase-B et/u tiles reuse the zone of the
    phase-A weights/x/rope temporaries)
"""

import sys
import numpy as np

for _p in ("/opt/trn_rl_repo",):
    if _p not in sys.path:
        sys.path.insert(0, _p)

import ml_dtypes  # noqa: E402

try:
    import antenv.axon_hooks  # noqa: F401
except ImportError:
    import types as _types

    _hooks = _types.ModuleType("antenv.axon_hooks")
    _hooks._hook = None
    _hooks.set_axon_ntff_profile_hook = lambda h: setattr(_hooks, "_hook", h)
    _hooks.get_axon_ntff_profile_hook = lambda: _hooks._hook
    sys.modules["antenv.axon_hooks"] = _hooks
    import antenv as _antenv

    _antenv.axon_hooks = _hooks
import concourse.bass as bass  # noqa: E402
import concourse.mybir as mybir  # noqa: E402
from concourse.bacc import Bacc  # noqa: E402
from concourse.tile import TileContext  # noqa: E402
from concourse.bass import ts, ds  # noqa: E402
from concourse.bass_utils import run_bass_kernel_spmd  # noqa: E402

B, T, C, H, D = 2, 2048, 2048, 16, 128
HL = 4                # heads per core
NCORES = 8
WINDOW = 512
ROPE_BASE = 10000.0
TB = 512              # i/t block size (matmul free dim)
NTB = T // TB         # 4
CK = C // 128         # 16 contraction chunks for projections
NTC = T // 128        # 16 j-chunks / t-chunks
MASK_OFF = 511        # trimmed mask strip: col = (i0 - j0) + MASK_OFF
MASK_W = 1664
F32 = mybir.dt.float32
BF16 = mybir.dt.bfloat16
AF = mybir.ActivationFunctionType

MM_DT = BF16
NP_MM = ml_dtypes.bfloat16

SKIP_MIN = 64         # only skip fully-masked i-strips at least this wide
SIM_SAFE = False      # memset skipped strips (needed only for CoreSim)
USE_DMA_SWAP = False  # fallback if cross-partition DVE operands are rejected

_NC = None
TRACE = False
LAST_RESULT = None


def _skip_strip(ib, c):
    """Fully-masked i-range (relative to block start) for j-chunk c in
    i-block ib: absolute i in [128c+127, 128c+511]."""
    lo = 128 * c + 127 - TB * ib
    hi = 128 * c + 512 - TB * ib
    lo, hi = max(lo, 0), min(hi, TB)
    if hi - lo >= SKIP_MIN:
        return lo, hi
    return None


def _mask_dd(ib, c):
    dd = ib * TB - c * 128
    if -(WINDOW - 1) <= dd <= (WINDOW - 1) + 127:
        return dd + MASK_OFF
    return None


def build_nc():
    nc = Bacc()
    xT = nc.declare_dram_parameter("xT", [C, T], MM_DT, isOutput=False)
    wq = nc.declare_dram_parameter("wq", [C, HL * D], MM_DT, isOutput=False)
    wk = nc.declare_dram_parameter("wk", [C, HL * D], MM_DT, isOutput=False)
    wv = nc.declare_dram_parameter("wv", [C, HL * D], MM_DT, isOutput=False)
    wo = nc.declare_dram_parameter("wo", [HL * D, C], MM_DT, isOutput=False)
    cosx = nc.declare_dram_parameter("cosx", [128, T], MM_DT, isOutput=False)
    sinx = nc.declare_dram_parameter("sinx", [128, T], MM_DT, isOutput=False)
    maskm = nc.declare_dram_parameter("maskm", [128, MASK_W], MM_DT, isOutput=False)
    out = nc.declare_dram_parameter("out", [T, C], MM_DT, isOutput=True)

    xT_v = xT[:].rearrange("(co p) t -> p co t", p=128)   # [128, 16, T]
    wq_v = wq[:].rearrange("(co p) d -> p co d", p=128)   # [128, 16, 512]
    wk_v = wk[:].rearrange("(co p) d -> p co d", p=128)
    wv_v = wv[:].rearrange("(co p) d -> p co d", p=128)
    wo_v = wo[:].rearrange("(h p) c -> p h c", p=128)     # [128, 4, C]

    scale = float(1.0 / np.sqrt(D))

    with TileContext(nc) as tc:
        with (
            tc.tile_pool(name="res", bufs=1) as res,
            tc.tile_pool(name="smp", bufs=2) as smp,
            tc.tile_pool(name="ocb", bufs=2) as ocb,
            tc.tile_pool(name="ps", bufs=1, space="PSUM") as psum,
        ):
            # ---- long-lived residents needed through phase B ----
            maskb = res.tile([128, MASK_W], MM_DT)
            wob = res.tile([128, HL, C], MM_DT)
            ones = res.tile([128, 128], MM_DT)
            qts = [res.tile([128, HL, TB], MM_DT, name=f"QT{t}") for t in range(NTB)]
            kts = [res.tile([128, HL, TB], MM_DT, name=f"KT{t}") for t in range(NTB)]
            V = res.tile([128, NTC, HL * D], MM_DT)
            oT = res.tile([128, HL, T], MM_DT)

            # 4 PSUM tile tags of [128, 2, TB] f32 (2 banks each), shared by
            # both phases.
            def ptile(i, name):
                return psum.tile([128, 2, TB], F32, tag=f"p{i}", name=name)

            # ---- Phase A: projections + RoPE (per-head chains), V ----
            with (
                tc.tile_pool(name="wp", bufs=1) as wp,
                tc.tile_pool(name="xbp", bufs=2) as xbp,
                tc.tile_pool(name="ropep", bufs=3) as ropep,
            ):
                # wk first, in ck-group chunks: the first QK chain can
                # start after the first chunk lands
                wkb = wp.tile([128, CK, HL * D], MM_DT)
                wqb = wp.tile([128, CK, HL * D], MM_DT)
                wvb = wp.tile([128, CK, HL * D], MM_DT)
                for a, b in ((0, 1), (1, 2), (2, 4), (4, 8), (8, 16)):
                    nc.sync.dma_start(wkb[:, a:b, :], wk_v[:, a:b, :])
                cosb = wp.tile([128, T], MM_DT)
                sinb = wp.tile([128, T], MM_DT)
                nc.sync.dma_start(cosb[:], cosx[:])
                nc.sync.dma_start(sinb[:], sinx[:])
                for g in range(4):
                    nc.sync.dma_start(wqb[:, ts(g, 4), :], wq_v[:, ts(g, 4), :])
                for g in range(2):
                    nc.sync.dma_start(wvb[:, ts(g, 8), :], wv_v[:, ts(g, 8), :])
                nc.sync.dma_start(maskb[:], maskm[:])
                nc.sync.dma_start(wob[:], wo_v[:])
                nc.gpsimd.memset(ones[:], 1.0)

                for tb in range(NTB):
                    xb = xbp.tile([128, CK, TB], MM_DT, tag="xb", name=f"xb{tb}")
                    groups = ((0, 1), (1, 2), (2, 4), (4, 8), (8, 16)) if tb == 0 \
                        else ((0, 4), (4, 8), (8, 12), (12, 16))
                    for a, b in groups:
                        nc.gpsimd.dma_start(
                            xb[:, a:b, :], xT_v[:, a:b, ts(tb, TB)]
                        )
                    for h in range(HL):
                        ph = ptile(h, f"pqk{tb}_{h}")
                        # K chain then Q chain (phase B scores of head h need
                        # K of the last tb first)
                        for half, wb in ((1, wkb), (0, wqb)):
                            for ck in range(CK):
                                nc.tensor.matmul(
                                    ph[:, half, :], wb[:, ck, ts(h, D)],
                                    xb[:, ck, :],
                                    start=(ck == 0), stop=(ck == CK - 1),
                                )
                        for half, OUTT in ((1, kts[tb]), (0, qts[tb])):
                            ps = ph[:, half, :]
                            if not USE_DMA_SWAP:
                                # RoPE on DVE reading PSUM directly:
                                #   out[0:64]   = ps[0:64]*cos - ps[64:128]*sin
                                #   out[64:128] = ps[64:128]*cos + ps[0:64]*sin
                                # sinx rows hold [-sin; sin]: both halves add.
                                tsw = ropep.tile([128, TB], F32, tag="tsw")
                                nc.vector.tensor_mul(
                                    tsw[0:64, :], ps[64:128, :],
                                    sinb[0:64, ts(tb, TB)],
                                )
                                nc.vector.tensor_mul(
                                    tsw[64:128, :], ps[0:64, :],
                                    sinb[64:128, ts(tb, TB)],
                                )
                                tcs = ropep.tile([128, TB], F32, tag="tcs")
                                nc.vector.tensor_mul(
                                    tcs[:], ps[:], cosb[:, ts(tb, TB)]
                                )
                                nc.vector.tensor_add(
                                    OUTT[:, h, :], tsw[:], tcs[:]
                                )
                            else:
                                sw = ropep.tile([128, TB], F32, tag="tsw")
                                nc.scalar.dma_start(sw[0:64, :], ps[64:128, :])
                                nc.scalar.dma_start(sw[64:128, :], ps[0:64, :])
                                nc.vector.tensor_mul(
                                    sw[:], sw[:], sinb[:, ts(tb, TB)]
                                )
                                tcs = ropep.tile([128, TB], F32, tag="tcs")
                                nc.vector.tensor_mul(
                                    tcs[:], ps[:], cosb[:, ts(tb, TB)]
                                )
                                nc.vector.tensor_add(OUTT[:, h, :], sw[:], tcs[:])
                    # V for the 4 t-chunks of this t-block
                    for tco in range(NTB):
                        tch = tb * NTB + tco
                        pv = ptile(tco // 2, f"pv{tb}_{tco}")[:, tco % 2, :]
                        for ck in range(CK):
                            nc.tensor.matmul(
                                pv[:], xb[:, ck, ts(tco, 128)], wvb[:, ck, :],
                                start=(ck == 0), stop=(ck == CK - 1),
                            )
                        nc.scalar.copy(V[:, tch, :], pv[:])

            # ---- Phase B: attention, with O-proj of ib-1 interleaved ----
            with (
                tc.tile_pool(name="etp", bufs=10) as etp,
                tc.tile_pool(name="up", bufs=2) as up,
            ):
                def oproj_group(ibo, cb, tags=(3,)):
                    for tto in range(NTB):
                        tt = ibo * NTB + tto
                        tg = tags[tto % len(tags)]
                        pc = ptile(tg, f"po{ibo}_{cb}_{tto}")[:, (tto // len(tags)) % 2, :]
                        for hh in range(HL):
                            nc.tensor.matmul(
                                pc[:], oT[:, hh, ts(tt, 128)],
                                wob[:, hh, ds(cb * TB, TB)],
                                start=(hh == 0), stop=(hh == HL - 1),
                            )
                        ob = ocb.tile([128, TB], MM_DT, tag=f"ob{tto % 2}")
                        if tto % 2 == 0:
                            nc.scalar.copy(ob[:], pc[:])
                        else:
                            nc.vector.tensor_copy(ob[:], pc[:])
                        nc.sync.dma_start(
                            out[ts(tt, 128), ds(cb * TB, TB)], ob[:]
                        )

                for ib in range(NTB):
                    for h in range(HL):
                        ets = []
                        u = up.tile([128, NTC // 2, TB], MM_DT, tag="u",
                                    name=f"u{ib}_{h}")
                        pso = ptile(2, f"pso{ib}_{h}")
                        for cp in range(NTC // 2):
                            # ib 0 has no O-proj filler: use its idle p3 tag
                            # as a third scores buffer to ride out exp lag
                            stag = (cp % 3 if cp % 3 < 2 else 3) if ib == 0 \
                                else cp % 2
                            sp = ptile(stag, f"ps{ib}_{h}_{cp}")
                            for k in range(2):
                                c = 2 * cp + k
                                kt = kts[c // 4][:, h, ts(c % 4, 128)]
                                strip = _skip_strip(ib, c)
                                if strip is None:
                                    nc.tensor.matmul(
                                        sp[:, k, :], kt, qts[ib][:, h, :],
                                        start=True, stop=True,
                                    )
                                else:
                                    lo, hi = strip
                                    if SIM_SAFE:
                                        # defined data for CoreSim's race check;
                                        # on HW the stale strip is bounded and
                                        # the mask multiply zeroes it
                                        nc.vector.memset(sp[:, k, lo:hi], 0.0)
                                    first = True
                                    if lo > 0:
                                        nc.tensor.matmul(
                                            sp[:, k, 0:lo], kt,
                                            qts[ib][:, h, 0:lo],
                                            start=first, stop=True,
                                        )
                                        first = False
                                    if hi < TB:
                                        nc.tensor.matmul(
                                            sp[:, k, hi:TB], kt,
                                            qts[ib][:, h, hi:TB],
                                            start=first, stop=True,
                                        )
                            et = etp.tile([128, 2, TB], MM_DT, tag="et")
                            nc.scalar.activation(et[:], sp[:], AF.Exp, scale=scale)
                            for k in range(2):
                                c = 2 * cp + k
                                off = _mask_dd(ib, c)
                                if off is not None:
                                    nc.vector.tensor_mul(
                                        et[:, k, :], et[:, k, :],
                                        maskb[:, ds(off, TB)],
                                    )
                            eng = nc.gpsimd if cp < 4 else nc.vector
                            eng.tensor_add(
                                u[:, cp, :], et[:, 0, :], et[:, 1, :]
                            )
                            ets.append(et)
                            # interleave AV of pair cp-2 behind the scores:
                            # gives the PE work while exp catches up
                            if cp >= 2:
                                for c in (2 * cp - 4, 2 * cp - 3):
                                    nc.tensor.matmul(
                                        pso[:, 0, :], V[:, c, ts(h, D)],
                                        ets[c // 2][:, c % 2, :],
                                        start=(c == 0), stop=False,
                                    )
                        for c in range(2 * NTC // 2 - 4, NTC):
                            nc.tensor.matmul(
                                pso[:, 0, :], V[:, c, ts(h, D)],
                                ets[c // 2][:, c % 2, :],
                                start=False, stop=(c == NTC - 1),
                            )
                        # O-proj filler next: keeps the PE busy while the
                        # Z add tree drains (in-order engine queue)
                        if ib > 0:
                            oproj_group(ib - 1, h)
                        # Z: add tree 8 -> 1 on vector, then ones-matmul
                        for k in range(4):
                            nc.vector.tensor_add(
                                u[:, k, :], u[:, k, :], u[:, k + 4, :]
                            )
                        nc.vector.tensor_add(u[:, 0, :], u[:, 0, :], u[:, 1, :])
                        nc.vector.tensor_add(u[:, 2, :], u[:, 2, :], u[:, 3, :])
                        nc.vector.tensor_add(u[:, 0, :], u[:, 0, :], u[:, 2, :])
                        nc.tensor.matmul(
                            pso[:, 1, :], ones[:], u[:, 0, :],
                            start=True, stop=True,
                        )
                        rz = smp.tile([128, TB], F32, tag="rz")
                        nc.vector.reciprocal_approx_fast(rz[:], pso[:, 1, :])
                        nc.vector.tensor_mul(
                            oT[:, h, ts(ib, TB)], pso[:, 0, :], rz[:]
                        )
                for cb in range(NTB):
                    oproj_group(NTB - 1, cb, tags=(0, 1, 2, 3))

    nc.finalize()
    return nc


def _host_tables():
    inv_freq = (
        1.0 / (np.float32(ROPE_BASE) ** (np.arange(0, D, 2, dtype=np.float32) / np.float32(D)))
    ).astype(np.float32)
    t = np.arange(T, dtype=np.float32)
    freqs = (t[:, None] * inv_freq[None, :]).astype(np.float32)  # [T, 64]
    cos = np.cos(freqs).T.astype(np.float32)                     # [64, T]
    sin = np.sin(freqs).T.astype(np.float32)
    cosx = np.ascontiguousarray(np.concatenate([cos, cos], axis=0)).astype(NP_MM)
    sinx = np.ascontiguousarray(np.concatenate([-sin, sin], axis=0)).astype(NP_MM)
    p = np.arange(128, dtype=np.int64)[:, None]
    u = np.arange(MASK_W, dtype=np.int64)[None, :]
    delta = u - MASK_OFF - p          # = i - j for tile offset
    allow = ~((delta >= 0) & (delta <= WINDOW - 1))
    maskm = np.ascontiguousarray(allow.astype(NP_MM))
    return cosx, sinx, maskm


def make_in_maps(x, Wq, Wk, Wv, Wo):
    cosx, sinx, maskm = _host_tables()
    in_maps = []
    for core in range(NCORES):
        b, hg = divmod(core, NCORES // B)
        sl = slice(hg * HL * D, (hg + 1) * HL * D)
        in_maps.append(
            {
                "xT": np.ascontiguousarray(x[b].T.astype(NP_MM)),
                "wq": np.ascontiguousarray(Wq[:, sl].astype(NP_MM)),
                "wk": np.ascontiguousarray(Wk[:, sl].astype(NP_MM)),
                "wv": np.ascontiguousarray(Wv[:, sl].astype(NP_MM)),
                "wo": np.ascontiguousarray(Wo[sl, :].astype(NP_MM)),
                "cosx": cosx,
                "sinx": sinx,
                "maskm": maskm,
            }
        )
    return in_maps


def kernel(x, Wq, Wk, Wv, Wo):
    global _NC, LAST_RESULT
    if _NC is None:
        _NC = build_nc()
    x = np.asarray(x, dtype=np.float32)
    Wq = np.asarray(Wq, dtype=np.float32)
    Wk = np.asarray(Wk, dtype=np.float32)
    Wv = np.asarray(Wv, dtype=np.float32)
    Wo = np.asarray(Wo, dtype=np.float32)
    in_maps = make_in_maps(x, Wq, Wk, Wv, Wo)
    res = run_bass_kernel_spmd(_NC, in_maps, list(range(NCORES)), trace=TRACE)
    LAST_RESULT = res
    out = np.zeros((B, T, C), dtype=np.float32)
    for core in range(NCORES):
        b = core // (NCORES // B)
        out[b] += res.results[core]["out"].astype(np.float32)
    return out


# revision 5
# speedup vs baseline: 1.3674x; 1.1797x over previous
"""Trainium2 Bass kernel v2 for windowed (inverted-window) attention.

Problem: B=2, T=2048, C=2048, H=16 heads, D=128, WINDOW=512.
  q,k,v = x@Wq, x@Wk, x@Wv  (per-head reshape), RoPE on q,k,
  scores masked so positions INSIDE the causal window are masked out
  (attend only to j>i or j<i-511), softmax, o@Wo.

Sharding: 8 cores = 2 (batch) x 4 (head groups of 4 heads).
Each core computes its batch's 4 heads end-to-end plus a partial
output projection (row-chunk of Wo); host sums the 4 partials per batch.

v2 design vs v1:
  - per-head sequential QK accumulation chains; RoPE of head h overlaps
    chain h+1 (removes the serialized RoPE tail at the A->B boundary)
  - RoPE done by DVE reading PSUM directly with cross-partition operand
    slices (no scalar copy, no DMA half-swaps)
  - one PSUM pool with [128,2,512] tags shared across both phases
    (bank-granular start=True zeroing makes half-tile sharing safe)
  - scores matmuls skip the fully-masked i-strip of diagonal j-chunks
    (exp stays dense; mask multiply zeroes the stale strip)
  - output projection of block ib-1 interleaved as PE filler during ib;
    AV matmuls interleaved behind the score matmuls (exp-lag filler)
  - Z: gpsimd/vector pair-sums + vector add tree + single ones-matmul
  - bf16 output stores on the sync DMA queue; host accumulates fp32
  - phase-scoped SBUF pools (phase-B et/u tiles reuse the zone of the
    phase-A weights/x/rope temporaries)
  - ib=0 uses the idle O-proj PSUM tag as a third scores buffer
"""

import sys
import numpy as np

for _p in ("/opt/trn_rl_repo",):
    if _p not in sys.path:
        sys.path.insert(0, _p)

import ml_dtypes  # noqa: E402

try:
    import antenv.axon_hooks  # noqa: F401
except ImportError:
    import types as _types

    _hooks = _types.ModuleType("antenv.axon_hooks")
    _hooks._hook = None
    _hooks.set_axon_ntff_profile_hook = lambda h: setattr(_hooks, "_hook", h)
    _hooks.get_axon_ntff_profile_hook = lambda: _hooks._hook
    sys.modules["antenv.axon_hooks"] = _hooks
    import antenv as _antenv

    _antenv.axon_hooks = _hooks
import concourse.bass as bass  # noqa: E402
import concourse.mybir as mybir  # noqa: E402
from concourse.bacc import Bacc  # noqa: E402
from concourse.tile import TileContext  # noqa: E402
from concourse.bass import ts, ds  # noqa: E402
from concourse.bass_utils import run_bass_kernel_spmd  # noqa: E402

B, T, C, H, D = 2, 2048, 2048, 16, 128
HL = 4                # heads per core
NCORES = 8
WINDOW = 512
ROPE_BASE = 10000.0
TB = 512              # i/t block size (matmul free dim)
NTB = T // TB         # 4
CK = C // 128         # 16 contraction chunks for projections
NTC = T // 128        # 16 j-chunks / t-chunks
MASK_OFF = 511        # trimmed mask strip: col = (i0 - j0) + MASK_OFF
MASK_W = 1664
F32 = mybir.dt.float32
BF16 = mybir.dt.bfloat16
AF = mybir.ActivationFunctionType

MM_DT = BF16
NP_MM = ml_dtypes.bfloat16

SKIP_MIN = 64         # only skip fully-masked i-strips at least this wide
SIM_SAFE = False      # memset skipped strips (needed only for CoreSim)
USE_DMA_SWAP = False  # fallback if cross-partition DVE operands are rejected

_NC = None
TRACE = False
LAST_RESULT = None


def _skip_strip(ib, c):
    """Fully-masked i-range (relative to block start) for j-chunk c in
    i-block ib: absolute i in [128c+127, 128c+511]."""
    lo = 128 * c + 127 - TB * ib
    hi = 128 * c + 512 - TB * ib
    lo, hi = max(lo, 0), min(hi, TB)
    if hi - lo >= SKIP_MIN:
        return lo, hi
    return None


def _mask_dd(ib, c):
    dd = ib * TB - c * 128
    if -(WINDOW - 1) <= dd <= (WINDOW - 1) + 127:
        return dd + MASK_OFF
    return None


def build_nc():
    nc = Bacc()
    xT = nc.declare_dram_parameter("xT", [C, T], MM_DT, isOutput=False)
    wq = nc.declare_dram_parameter("wq", [C, HL * D], MM_DT, isOutput=False)
    wk = nc.declare_dram_parameter("wk", [C, HL * D], MM_DT, isOutput=False)
    wv = nc.declare_dram_parameter("wv", [C, HL * D], MM_DT, isOutput=False)
    wo = nc.declare_dram_parameter("wo", [HL * D, C], MM_DT, isOutput=False)
    cosx = nc.declare_dram_parameter("cosx", [128, T], MM_DT, isOutput=False)
    sinx = nc.declare_dram_parameter("sinx", [128, T], MM_DT, isOutput=False)
    maskm = nc.declare_dram_parameter("maskm", [128, MASK_W], MM_DT, isOutput=False)
    out = nc.declare_dram_parameter("out", [T, C], MM_DT, isOutput=True)

    xT_v = xT[:].rearrange("(co p) t -> p co t", p=128)   # [128, 16, T]
    wq_v = wq[:].rearrange("(co p) d -> p co d", p=128)   # [128, 16, 512]
    wk_v = wk[:].rearrange("(co p) d -> p co d", p=128)
    wv_v = wv[:].rearrange("(co p) d -> p co d", p=128)
    wo_v = wo[:].rearrange("(h p) c -> p h c", p=128)     # [128, 4, C]

    scale = float(1.0 / np.sqrt(D))

    with TileContext(nc) as tc:
        with (
            tc.tile_pool(name="res", bufs=1) as res,
            tc.tile_pool(name="smp", bufs=2) as smp,
            tc.tile_pool(name="ocb", bufs=2) as ocb,
            tc.tile_pool(name="ps", bufs=1, space="PSUM") as psum,
        ):
            # ---- long-lived residents needed through phase B ----
            maskb = res.tile([128, MASK_W], MM_DT)
            wob = res.tile([128, HL, C], MM_DT)
            ones = res.tile([128, 128], MM_DT)
            qts = [res.tile([128, HL, TB], MM_DT, name=f"QT{t}") for t in range(NTB)]
            kts = [res.tile([128, HL, TB], MM_DT, name=f"KT{t}") for t in range(NTB)]
            V = res.tile([128, NTC, HL * D], MM_DT)
            oT = res.tile([128, HL, T], MM_DT)

            # 4 PSUM tile tags of [128, 2, TB] f32 (2 banks each), shared by
            # both phases.
            def ptile(i, name):
                return psum.tile([128, 2, TB], F32, tag=f"p{i}", name=name)

            # ---- Phase A: projections + RoPE (per-head chains), V ----
            with (
                tc.tile_pool(name="wp", bufs=1) as wp,
                tc.tile_pool(name="xbp", bufs=2) as xbp,
                tc.tile_pool(name="ropep", bufs=3) as ropep,
            ):
                # wk first, in ck-group chunks: the first QK chain can
                # start after the first chunk lands
                wkb = wp.tile([128, CK, HL * D], MM_DT)
                wqb = wp.tile([128, CK, HL * D], MM_DT)
                wvb = wp.tile([128, CK, HL * D], MM_DT)
                for a, b in ((0, 1), (1, 2), (2, 4), (4, 8), (8, 16)):
                    nc.sync.dma_start(wkb[:, a:b, :], wk_v[:, a:b, :])
                cosb = wp.tile([128, T], MM_DT)
                sinb = wp.tile([128, T], MM_DT)
                nc.sync.dma_start(cosb[:], cosx[:])
                nc.sync.dma_start(sinb[:], sinx[:])
                for g in range(4):
                    nc.sync.dma_start(wqb[:, ts(g, 4), :], wq_v[:, ts(g, 4), :])
                for g in range(2):
                    nc.sync.dma_start(wvb[:, ts(g, 8), :], wv_v[:, ts(g, 8), :])
                nc.sync.dma_start(maskb[:], maskm[:])
                nc.sync.dma_start(wob[:], wo_v[:])
                nc.gpsimd.memset(ones[:], 1.0)

                for tb in range(NTB):
                    xb = xbp.tile([128, CK, TB], MM_DT, tag="xb", name=f"xb{tb}")
                    groups = ((0, 1), (1, 2), (2, 4), (4, 8), (8, 16)) if tb == 0 \
                        else ((0, 4), (4, 8), (8, 12), (12, 16))
                    for a, b in groups:
                        nc.gpsimd.dma_start(
                            xb[:, a:b, :], xT_v[:, a:b, ts(tb, TB)]
                        )
                    for h in range(HL):
                        ph = ptile(h, f"pqk{tb}_{h}")
                        # K chain then Q chain (phase B scores of head h need
                        # K of the last tb first)
                        for half, wb in ((1, wkb), (0, wqb)):
                            for ck in range(CK):
                                nc.tensor.matmul(
                                    ph[:, half, :], wb[:, ck, ts(h, D)],
                                    xb[:, ck, :],
                                    start=(ck == 0), stop=(ck == CK - 1),
                                )
                        for half, OUTT in ((1, kts[tb]), (0, qts[tb])):
                            ps = ph[:, half, :]
                            if not USE_DMA_SWAP:
                                # RoPE on DVE reading PSUM directly:
                                #   out[0:64]   = ps[0:64]*cos - ps[64:128]*sin
                                #   out[64:128] = ps[64:128]*cos + ps[0:64]*sin
                                # sinx rows hold [-sin; sin]: both halves add.
                                tsw = ropep.tile([128, TB], F32, tag="tsw")
                                nc.vector.tensor_mul(
                                    tsw[0:64, :], ps[64:128, :],
                                    sinb[0:64, ts(tb, TB)],
                                )
                                nc.vector.tensor_mul(
                                    tsw[64:128, :], ps[0:64, :],
                                    sinb[64:128, ts(tb, TB)],
                                )
                                tcs = ropep.tile([128, TB], F32, tag="tcs")
                                nc.vector.tensor_mul(
                                    tcs[:], ps[:], cosb[:, ts(tb, TB)]
                                )
                                nc.vector.tensor_add(
                                    OUTT[:, h, :], tsw[:], tcs[:]
                                )
                            else:
                                sw = ropep.tile([128, TB], F32, tag="tsw")
                                nc.scalar.dma_start(sw[0:64, :], ps[64:128, :])
                                nc.scalar.dma_start(sw[64:128, :], ps[0:64, :])
                                nc.vector.tensor_mul(
                                    sw[:], sw[:], sinb[:, ts(tb, TB)]
                                )
                                tcs = ropep.tile([128, TB], F32, tag="tcs")
                                nc.vector.tensor_mul(
                                    tcs[:], ps[:], cosb[:, ts(tb, TB)]
                                )
                                nc.vector.tensor_add(OUTT[:, h, :], sw[:], tcs[:])
                    # V for the 4 t-chunks of this t-block
                    for tco in range(NTB):
                        tch = tb * NTB + tco
                        pv = ptile(tco // 2, f"pv{tb}_{tco}")[:, tco % 2, :]
                        for ck in range(CK):
                            nc.tensor.matmul(
                                pv[:], xb[:, ck, ts(tco, 128)], wvb[:, ck, :],
                                start=(ck == 0), stop=(ck == CK - 1),
                            )
                        nc.scalar.copy(V[:, tch, :], pv[:])

            # ---- Phase B: attention, with O-proj of ib-1 interleaved ----
            with (
                tc.tile_pool(name="etp", bufs=10) as etp,
                tc.tile_pool(name="up", bufs=2) as up,
            ):
                def oproj_group(ibo, cb, tags=(3,)):
                    for tto in range(NTB):
                        tt = ibo * NTB + tto
                        tg = tags[tto % len(tags)]
                        pc = ptile(tg, f"po{ibo}_{cb}_{tto}")[:, (tto // len(tags)) % 2, :]
                        for hh in range(HL):
                            nc.tensor.matmul(
                                pc[:], oT[:, hh, ts(tt, 128)],
                                wob[:, hh, ds(cb * TB, TB)],
                                start=(hh == 0), stop=(hh == HL - 1),
                            )
                        ob = ocb.tile([128, TB], MM_DT, tag=f"ob{tto % 2}")
                        if tto % 2 == 0:
                            nc.scalar.copy(ob[:], pc[:])
                        else:
                            nc.vector.tensor_copy(ob[:], pc[:])
                        nc.sync.dma_start(
                            out[ts(tt, 128), ds(cb * TB, TB)], ob[:]
                        )

                for ib in range(NTB):
                    for h in range(HL):
                        ets = []
                        u = up.tile([128, NTC // 2, TB], MM_DT, tag="u",
                                    name=f"u{ib}_{h}")
                        pso = ptile(2, f"pso{ib}_{h}")
                        for cp in range(NTC // 2):
                            # ib 0 has no O-proj filler: use its idle p3 tag
                            # as a third scores buffer to ride out exp lag
                            stag = (cp % 3 if cp % 3 < 2 else 3) if ib == 0 \
                                else cp % 2
                            sp = ptile(stag, f"ps{ib}_{h}_{cp}")
                            for k in range(2):
                                c = 2 * cp + k
                                kt = kts[c // 4][:, h, ts(c % 4, 128)]
                                strip = _skip_strip(ib, c)
                                if strip is None:
                                    nc.tensor.matmul(
                                        sp[:, k, :], kt, qts[ib][:, h, :],
                                        start=True, stop=True,
                                    )
                                else:
                                    lo, hi = strip
                                    if SIM_SAFE:
                                        # defined data for CoreSim's race check;
                                        # on HW the stale strip is bounded and
                                        # the mask multiply zeroes it
                                        nc.vector.memset(sp[:, k, lo:hi], 0.0)
                                    first = True
                                    if lo > 0:
                                        nc.tensor.matmul(
                                            sp[:, k, 0:lo], kt,
                                            qts[ib][:, h, 0:lo],
                                            start=first, stop=True,
                                        )
                                        first = False
                                    if hi < TB:
                                        nc.tensor.matmul(
                                            sp[:, k, hi:TB], kt,
                                            qts[ib][:, h, hi:TB],
                                            start=first, stop=True,
                                        )
                            et = etp.tile([128, 2, TB], MM_DT, tag="et")
                            nc.scalar.activation(et[:], sp[:], AF.Exp, scale=scale)
                            for k in range(2):
                                c = 2 * cp + k
                                off = _mask_dd(ib, c)
                                if off is not None:
                                    nc.vector.tensor_mul(
                                        et[:, k, :], et[:, k, :],
                                        maskb[:, ds(off, TB)],
                                    )
                            eng = nc.gpsimd if cp < 4 else nc.vector
                            eng.tensor_add(
                                u[:, cp, :], et[:, 0, :], et[:, 1, :]
                            )
                            ets.append(et)
                            # interleave AV of pair cp-2 behind the scores:
                            # gives the PE work while exp catches up
                            if cp >= 2:
                                for c in (2 * cp - 4, 2 * cp - 3):
                                    nc.tensor.matmul(
                                        pso[:, 0, :], V[:, c, ts(h, D)],
                                        ets[c // 2][:, c % 2, :],
                                        start=(c == 0), stop=False,
                                    )
                        for c in range(2 * NTC // 2 - 4, NTC):
                            nc.tensor.matmul(
                                pso[:, 0, :], V[:, c, ts(h, D)],
                                ets[c // 2][:, c % 2, :],
                                start=False, stop=(c == NTC - 1),
                            )
                        # O-proj filler next: keeps the PE busy while the
                        # Z add tree drains (in-order engine queue)
                        if ib > 0:
                            oproj_group(ib - 1, h)
                        # Z: add tree 8 -> 1 on vector, then ones-matmul
                        for k in range(4):
                            nc.vector.tensor_add(
                                u[:, k, :], u[:, k, :], u[:, k + 4, :]
                            )
                        nc.vector.tensor_add(u[:, 0, :], u[:, 0, :], u[:, 1, :])
                        nc.vector.tensor_add(u[:, 2, :], u[:, 2, :], u[:, 3, :])
                        nc.vector.tensor_add(u[:, 0, :], u[:, 0, :], u[:, 2, :])
                        nc.tensor.matmul(
                            pso[:, 1, :], ones[:], u[:, 0, :],
                            start=True, stop=True,
                        )
                        rz = smp.tile([128, TB], F32, tag="rz")
                        nc.vector.reciprocal_approx_fast(rz[:], pso[:, 1, :])
                        nc.vector.tensor_mul(
                            oT[:, h, ts(ib, TB)], pso[:, 0, :], rz[:]
                        )
                for cb in range(NTB):
                    oproj_group(NTB - 1, cb, tags=(0, 1, 2, 3))

    nc.finalize()
    return nc


def _host_tables():
    inv_freq = (
        1.0 / (np.float32(ROPE_BASE) ** (np.arange(0, D, 2, dtype=np.float32) / np.float32(D)))
    ).astype(np.float32)
    t = np.arange(T, dtype=np.float32)
    freqs = (t[:, None] * inv_freq[None, :]).astype(np.float32)  # [T, 64]
    cos = np.cos(freqs).T.astype(np.float32)                     # [64, T]
    sin = np.sin(freqs).T.astype(np.float32)
    cosx = np.ascontiguousarray(np.concatenate([cos, cos], axis=0)).astype(NP_MM)
    sinx = np.ascontiguousarray(np.concatenate([-sin, sin], axis=0)).astype(NP_MM)
    p = np.arange(128, dtype=np.int64)[:, None]
    u = np.arange(MASK_W, dtype=np.int64)[None, :]
    delta = u - MASK_OFF - p          # = i - j for tile offset
    allow = ~((delta >= 0) & (delta <= WINDOW - 1))
    maskm = np.ascontiguousarray(allow.astype(NP_MM))
    return cosx, sinx, maskm


def make_in_maps(x, Wq, Wk, Wv, Wo):
    cosx, sinx, maskm = _host_tables()
    in_maps = []
    for core in range(NCORES):
        b, hg = divmod(core, NCORES // B)
        sl = slice(hg * HL * D, (hg + 1) * HL * D)
        in_maps.append(
            {
                "xT": np.ascontiguousarray(x[b].T.astype(NP_MM)),
                "wq": np.ascontiguousarray(Wq[:, sl].astype(NP_MM)),
                "wk": np.ascontiguousarray(Wk[:, sl].astype(NP_MM)),
                "wv": np.ascontiguousarray(Wv[:, sl].astype(NP_MM)),
                "wo": np.ascontiguousarray(Wo[sl, :].astype(NP_MM)),
                "cosx": cosx,
                "sinx": sinx,
                "maskm": maskm,
            }
        )
    return in_maps


def kernel(x, Wq, Wk, Wv, Wo):
    global _NC, LAST_RESULT
    if _NC is None:
        _NC = build_nc()
    x = np.asarray(x, dtype=np.float32)
    Wq = np.asarray(Wq, dtype=np.float32)
    Wk = np.asarray(Wk, dtype=np.float32)
    Wv = np.asarray(Wv, dtype=np.float32)
    Wo = np.asarray(Wo, dtype=np.float32)
    in_maps = make_in_maps(x, Wq, Wk, Wv, Wo)
    res = run_bass_kernel_spmd(_NC, in_maps, list(range(NCORES)), trace=TRACE)
    LAST_RESULT = res
    out = np.zeros((B, T, C), dtype=np.float32)
    for core in range(NCORES):
        b = core // (NCORES // B)
        out[b] += res.results[core]["out"].astype(np.float32)
    return out
